# revision 16
# baseline (speedup 1.0000x reference)
"""Trainium2 Bass kernel for nn_MultiHeadAttention_5695126634518.

Computes (out, weights) for the quirky MHA variant:
  q/k/v = per-head projections of x; logits[b,h,i,j] = q_j . k_i / sqrt(C);
  weights = causal softmax(logits) (full [B,H,T,T] returned);
  attn = diag(weights) * v; out = concat(attn) @ Wp.T + bp.

Sharding: 8 cores = 4 batches x 2 head-groups (8 heads each).
Each core computes its group's weights [8,T,T] (lower triangle only --
ExternalOutput buffers are pre-zeroed) and a partial out projection
[T,C]; host sums the two per-batch partials and adds the bias.

All matmuls run in fp16 (1 cycle/row on the PE vs 4 for fp32) with fp32
PSUM accumulation; softmax statistics and the weights output stay fp32.
"""

import os
import sys

sys.path.insert(0, "/opt/trn_rl_repo")

_PHASES = os.environ.get("K_PHASES", "1234")

from contextlib import ExitStack

import numpy as np

import concourse.bacc as bacc
import concourse.tile as tile
from concourse import mybir
from concourse.bass_utils import run_bass_kernel_spmd

B, T, C, H = 4, 2048, 1024, 16
HD = C // H           # 64
G = 2                 # head groups (cores per batch)
HL = H // G           # 8 heads per core
NP = HL // 2          # 4 head pairs (two heads packed per 128 partitions)
NCHUNK = C // 128     # 8 contraction chunks
NT = T // 512         # 4 moving tiles of 512
SCALE = 1.0 / float(np.sqrt(C))
N_CORES = 8

f16 = mybir.dt.float16
f32 = mybir.dt.float32
EXP = mybir.ActivationFunctionType.Exp
COPY = mybir.ActivationFunctionType.Copy
MULT = mybir.AluOpType.mult
ADD = mybir.AluOpType.add
AXX = mybir.AxisListType.X

_CACHE = {}


def _build():
    nc = bacc.Bacc("TRN2", target_bir_lowering=False, debug=False)

    xT = nc.dram_tensor("xT", [NCHUNK, 128, T], f16, kind="ExternalInput")
    wq = nc.dram_tensor("wq", [NCHUNK, 128, HL * HD], f16, kind="ExternalInput")
    wk = nc.dram_tensor("wk", [NCHUNK, 128, HL * HD], f16, kind="ExternalInput")
    wv = nc.dram_tensor("wv", [NCHUNK, 128, HL * HD], f16, kind="ExternalInput")
    wp = nc.dram_tensor("wp", [NP, 128, C], f16, kind="ExternalInput")
    tri = nc.dram_tensor("tri", [128, 128], f32, kind="ExternalInput")
    idn = nc.dram_tensor("idn", [128, 128], f32, kind="ExternalInput")
    w_out = nc.dram_tensor("w_out", [HL, T, T], f32, kind="ExternalOutput")
    o_out = nc.dram_tensor("o_out", [T, C], f32, kind="ExternalOutput")

    with tile.TileContext(nc) as tc, ExitStack() as ctx:
        const = ctx.enter_context(tc.tile_pool(name="const", bufs=1))
        big = ctx.enter_context(tc.tile_pool(name="big", bufs=1))
        wpool = ctx.enter_context(tc.tile_pool(name="wpool", bufs=2))
        stage = ctx.enter_context(tc.tile_pool(name="stage", bufs=6))
        small = ctx.enter_context(tc.tile_pool(name="small", bufs=4))
        psA = ctx.enter_context(tc.tile_pool(name="psA", bufs=3, space="PSUM"))
        psMM = ctx.enter_context(tc.tile_pool(name="psMM", bufs=2, space="PSUM"))
        psD = ctx.enter_context(tc.tile_pool(name="psD", bufs=1, space="PSUM"))

        # ---- load inputs
        xT_sb = big.tile([128, NCHUNK, T], f16)
        for c in range(NCHUNK):
            nc.sync.dma_start(out=xT_sb[:, c, :], in_=xT[c])
        w_sbs = []
        for name, src in (("wq", wq), ("wk", wk), ("wv", wv)):
            sb = big.tile([128, NCHUNK, HL * HD], f16, tag=name)
            for c in range(NCHUNK):
                nc.sync.dma_start(out=sb[:, c, :], in_=src[c])
            w_sbs.append(sb)
        wq_sb, wk_sb, wv_sb = w_sbs
        wp_sb = big.tile([128, NP, C], f16)
        for kc in range(NP):
            nc.sync.dma_start(out=wp_sb[:, kc, :], in_=wp[kc])
        tri_sb = const.tile([128, 128], f32)
        nc.sync.dma_start(out=tri_sb, in_=tri[:, :])
        idn_sb = const.tile([128, 128], f32)
        nc.sync.dma_start(out=idn_sb, in_=idn[:, :])
        idn16_sb = const.tile([128, 128], f16)
        nc.vector.tensor_copy(out=idn16_sb, in_=idn_sb)
        ones16 = const.tile([1, 64], f16)
        nc.vector.memset(ones16, 1.0)

        # ---- persistent intermediates
        qT_sb = big.tile([128, NP, T], f16)
        kT_sb = big.tile([128, NP, T], f16)
        vT_sb = big.tile([128, NP, T], f16)   # becomes catT in place
        S_sb = big.tile([128, HL, 16], f32)   # row sums (column form)
        R_sb = big.tile([128, HL, 16], f32)   # 1/S (column form)
        E_sb = big.tile([128, HL, 16], f16)   # diag weights (column form)
        rf_row = big.tile([1, T], f16)        # diag weights, row form (per head)

        # ---- phase 1: q/k/v projections, two heads packed per matmul
        for wsb, dst in ((wq_sb, qT_sb), (wk_sb, kT_sb), (wv_sb, vT_sb)):
            for p in range(NP):
                for jt in range(NT):
                    ps = psMM.tile([128, 512], f32, tag="mm")
                    for c in range(NCHUNK):
                        nc.tensor.matmul(
                            ps,
                            wsb[:, c, p * 128 : (p + 1) * 128],
                            xT_sb[:, c, jt * 512 : (jt + 1) * 512],
                            start=(c == 0),
                            stop=(c == NCHUNK - 1),
                        )
                    nc.scalar.activation(
                        out=dst[:, p, jt * 512 : (jt + 1) * 512], in_=ps, func=COPY
                    )

        # ---- phases 2-4 per head: logits, exp, sums, normalize, write, catT
        SUB = os.environ.get("K_P2SUB", "e")
        for h in range(HL if "2" in _PHASES else 0):
            p, o = h // 2, (h % 2) * 64
            kTh = kT_sb[o : o + 64, p, :]
            qTh = qT_sb[o : o + 64, p, :]
            for i in range(16):
                edge = (i + 1) * 128
                nf, rem = divmod(edge, 512)
                nplain = nf - 1 if rem == 0 else nf
                wlast = 512 if rem == 0 else rem
                c0 = nplain * 512
                lead = wlast - 128
                kblk = kTh[:, i * 128 : edge]

                wt = wpool.tile([128, T], f32)
                st = small.tile([128, 8], f32)
                ncols = 0
                for jt in range(nplain):
                    ps = psA.tile([128, 512], f32)
                    nc.tensor.matmul(
                        ps, kblk, qTh[:, jt * 512 : (jt + 1) * 512],
                        start=True, stop=True,
                    )
                    if SUB >= "b":
                        nc.scalar.activation(
                            out=wt[:, jt * 512 : (jt + 1) * 512], in_=ps, func=EXP,
                            scale=SCALE, accum_out=st[:, ncols : ncols + 1],
                        )
                    else:
                        nc.scalar.activation(
                            out=wt[:, jt * 512 : (jt + 1) * 512], in_=ps, func=EXP,
                            scale=SCALE,
                        )
                    ncols += 1
                ps = psA.tile([128, 512], f32)
                nc.tensor.matmul(
                    ps[:, :wlast], kblk, qTh[:, c0:edge], start=True, stop=True
                )
                if lead:
                    if SUB >= "b":
                        nc.scalar.activation(
                            out=wt[:, c0 : c0 + lead], in_=ps[:, :lead], func=EXP,
                            scale=SCALE, accum_out=st[:, ncols : ncols + 1],
                        )
                    else:
                        nc.scalar.activation(
                            out=wt[:, c0 : c0 + lead], in_=ps[:, :lead], func=EXP,
                            scale=SCALE,
                        )
                    ncols += 1
                dcol = c0 + lead  # == i * 128
                nc.scalar.activation(
                    out=wt[:, dcol : dcol + 128], in_=ps[:, lead:wlast], func=EXP,
                    scale=SCALE,
                )
                if SUB >= "c":
                    # diagonal extraction (pre-mask), then in-place causal mask
                    dump = small.tile([128, 128], f32)
                    ec = small.tile([128, 1], f32)
                    nc.vector.tensor_mul(dump, wt[:, dcol : dcol + 128], idn_sb)
                    nc.vector.tensor_reduce(out=ec, in_=dump, axis=AXX, op=ADD)
                    nc.vector.tensor_mul(
                        wt[:, dcol : dcol + 128], wt[:, dcol : dcol + 128], tri_sb
                    )
                    nc.vector.tensor_reduce(
                        out=st[:, ncols : ncols + 1], in_=wt[:, dcol : dcol + 128],
                        axis=AXX, op=ADD,
                    )
                    ncols += 1
                if SUB >= "d":
                    nc.vector.tensor_reduce(
                        out=S_sb[:, h, i : i + 1], in_=st[:, 0:ncols], axis=AXX, op=ADD
                    )
                    nc.vector.reciprocal(
                        out=R_sb[:, h, i : i + 1], in_=S_sb[:, h, i : i + 1]
                    )
                    nc.vector.tensor_mul(
                        E_sb[:, h, i : i + 1], ec, R_sb[:, h, i : i + 1]
                    )
                if SUB >= "e":
                    # normalize + write lower-triangle chunks
                    for jt in range(nplain):
                        sg = stage.tile([128, 512], f32)
                        nc.vector.tensor_scalar_mul(
                            sg, wt[:, jt * 512 : (jt + 1) * 512], R_sb[:, h, i : i + 1]
                        )
                        nc.sync.dma_start(
                            out=w_out[h, i * 128 : edge, jt * 512 : (jt + 1) * 512],
                            in_=sg,
                        )
                    sg = stage.tile([128, 512], f32)
                    nc.vector.tensor_scalar_mul(
                        sg[:, :wlast], wt[:, c0:edge], R_sb[:, h, i : i + 1]
                    )
                    nc.sync.dma_start(
                        out=w_out[h, i * 128 : edge, c0:edge], in_=sg[:, :wlast]
                    )
                elif SUB >= "a":
                    # legality probe: dump unnormalized tiles straight out
                    nc.sync.dma_start(
                        out=w_out[h, i * 128 : edge, 0:edge], in_=wt[:, :edge]
                    )

            if "3" not in _PHASES:
                continue
            # diag weights: column form -> row form -> broadcast -> catT
            pst = psD.tile([16, 128], f16, tag="pst")
            nc.tensor.transpose(pst, E_sb[:, h, :], idn16_sb)
            et = small.tile([16, 128], f16)
            nc.vector.tensor_copy(out=et, in_=pst)
            nc.sync.dma_start(
                out=rf_row.rearrange("p (b u) -> p b u", u=128), in_=et
            )
            for jt in range(NT):
                ps_bc = psD.tile([64, 512], f32, tag="bc")
                nc.tensor.matmul(
                    ps_bc, ones16, rf_row[0:1, jt * 512 : (jt + 1) * 512],
                    start=True, stop=True,
                )
                nc.vector.tensor_tensor(
                    out=vT_sb[o : o + 64, p, jt * 512 : (jt + 1) * 512],
                    in0=vT_sb[o : o + 64, p, jt * 512 : (jt + 1) * 512],
                    in1=ps_bc, op=MULT,
                )

        # ---- phase 5: output projection (partial over this core's heads)
        for i in range(16 if "4" in _PHASES else 0):
            for n in range(2):
                ps = psMM.tile([128, 512], f32, tag="mm")
                for kc in range(NP):
                    nc.tensor.matmul(
                        ps,
                        vT_sb[:, kc, i * 128 : (i + 1) * 128],
                        wp_sb[:, kc, n * 512 : (n + 1) * 512],
                        start=(kc == 0),
                        stop=(kc == NP - 1),
                    )
                sg = stage.tile([128, 512], f32)
                nc.scalar.activation(out=sg, in_=ps, func=COPY)
                nc.sync.dma_start(
                    out=o_out[i * 128 : (i + 1) * 128, n * 512 : (n + 1) * 512],
                    in_=sg,
                )

    nc.compile()
    return nc


def _get_nc():
    if "nc" not in _CACHE:
        _CACHE["nc"] = _build()
    return _CACHE["nc"]


_TRI = np.tril(np.ones((128, 128), np.float32))
_IDN = np.eye(128, dtype=np.float32)


def _pack_w(W, h0):
    # W [H, C, HD] -> [NCHUNK, 128, HL*HD] with cols = h_local*HD + d
    A = W[h0 : h0 + HL].transpose(1, 0, 2).reshape(C, HL * HD)
    return np.ascontiguousarray(A).reshape(NCHUNK, 128, HL * HD).astype(np.float16)


def make_in_maps(x, Wq, Wk, Wv, Wp):
    in_maps = []
    for core in range(N_CORES):
        b, g = core // G, core % G
        h0 = g * HL
        xb = np.ascontiguousarray(x[b].T).reshape(NCHUNK, 128, T).astype(np.float16)
        wp_pack = (
            np.ascontiguousarray(Wp[:, g * HL * HD : (g + 1) * HL * HD].T)
            .reshape(NP, 128, C)
            .astype(np.float16)
        )
        in_maps.append(
            {
                "xT": xb,
                "wq": _pack_w(Wq, h0),
                "wk": _pack_w(Wk, h0),
                "wv": _pack_w(Wv, h0),
                "wp": wp_pack,
                "tri": _TRI,
                "idn": _IDN,
            }
        )
    return in_maps


def assemble(results, bp):
    weights = np.empty((B, H, T, T), np.float32)
    out = np.empty((B, T, C), np.float32)
    for b in range(B):
        weights[b, :HL] = results[G * b]["w_out"]
        weights[b, HL:] = results[G * b + 1]["w_out"]
        out[b] = results[G * b]["o_out"] + results[G * b + 1]["o_out"] + bp[None, :]
    return out, weights


def kernel(**inputs):
    x = np.asarray(inputs["x"], np.float32)
    Wq = np.asarray(inputs["Wq"], np.float32)
    Wk = np.asarray(inputs["Wk"], np.float32)
    Wv = np.asarray(inputs["Wv"], np.float32)
    Wp = np.asarray(inputs["Wp"], np.float32)
    bp = np.asarray(inputs["bp"], np.float32)
    nc = _get_nc()
    in_maps = make_in_maps(x, Wq, Wk, Wv, Wp)
    res = run_bass_kernel_spmd(nc, in_maps, core_ids=list(range(N_CORES)))
    return assemble(res.results, bp)


# revision 29
# speedup vs baseline: 1.4655x; 1.4655x over previous
"""Trainium2 Bass kernel for nn_MultiHeadAttention_5695126634518.

Computes (out, weights) for the quirky MHA variant:
  q/k/v = per-head projections of x; logits[b,h,i,j] = q_j . k_i / sqrt(C);
  weights = causal softmax(logits) (full [B,H,T,T] returned);
  attn = diag(weights) * v; out = concat(attn) @ Wp.T + bp.

Sharding: 8 cores = 4 batches x 2 head-groups (8 heads each).
Each core computes its group's weights [8,T,T] (lower triangle only --
ExternalOutput buffers are pre-zeroed by the PJRT runner) and a partial
out projection [T,C]; host sums the two per-batch partials + bias.

Key structure (per core):
  - all matmuls fp16 (1 cycle/row on PE), fp32 PSUM accumulation
  - causal mask applied ON THE PE: a second matmul accumulates a
    [128,128] upper-triangular -60000 constant onto the logits PSUM of
    each diagonal block; exp then underflows those lanes to exactly 0
  - softmax: one ScalarE exp per [128,<=1024] PSUM slab with fused
    accum_out row-sums; per-band in-place VectorE normalize; one DMA
    per band row (issued alternately from SP and GpSimd queues)
  - diag(weights) comes from an elementwise q*k + ones-matmul reduction
    (row form), times 1/S transposed to row form via a PE transpose
  - attn^T is formed in place in vT via a rank-1 broadcast matmul of the
    diag weights; output projection contracts attn^T against Wp^T
"""

import os
import sys

sys.path.insert(0, "/opt/trn_rl_repo")

from contextlib import ExitStack

import numpy as np

import concourse.bacc as bacc
import concourse.tile as tile
from concourse import mybir
from concourse.bass_utils import run_bass_kernel_spmd

B, T, C, H = 4, 2048, 1024, 16
HD = C // H           # 64
G = 2                 # head groups (cores per batch)
HL = H // G           # 8 heads per core
NP = HL // 2          # 4 head pairs (two heads packed per 128 partitions)
NCHUNK = C // 128     # 8 contraction chunks
NT = T // 512         # 4 moving tiles of 512
SCALE = 1.0 / float(np.sqrt(C))
N_CORES = 8
NEG = -60000.0        # causal mask addend (fp16-representable; *SCALE -> exp==0)

f16 = mybir.dt.float16
f32 = mybir.dt.float32
EXP = mybir.ActivationFunctionType.Exp
COPY = mybir.ActivationFunctionType.Copy
MULT = mybir.AluOpType.mult
ADD = mybir.AluOpType.add
AXX = mybir.AxisListType.X

_CACHE = {}


def _build():
    nc = bacc.Bacc("TRN2", target_bir_lowering=False, debug=False)

    xT = nc.dram_tensor("xT", [NCHUNK, 128, T], f16, kind="ExternalInput")
    wq = nc.dram_tensor("wq", [NCHUNK, 128, HL * HD], f16, kind="ExternalInput")
    wk = nc.dram_tensor("wk", [NCHUNK, 128, HL * HD], f16, kind="ExternalInput")
    wv = nc.dram_tensor("wv", [NCHUNK, 128, HL * HD], f16, kind="ExternalInput")
    wp = nc.dram_tensor("wp", [NP, 128, C], f16, kind="ExternalInput")
    negm = nc.dram_tensor("negm", [128, 128], f16, kind="ExternalInput")
    idn16 = nc.dram_tensor("idn16", [128, 128], f16, kind="ExternalInput")
    idn32 = nc.dram_tensor("idn32", [128, 128], f32, kind="ExternalInput")
    w_out = nc.dram_tensor("w_out", [HL, T, T], f16, kind="ExternalOutput")
    o_out = nc.dram_tensor("o_out", [T, C], f32, kind="ExternalOutput")

    with tile.TileContext(nc) as tc, ExitStack() as ctx:
        const = ctx.enter_context(tc.tile_pool(name="const", bufs=1))
        big = ctx.enter_context(tc.tile_pool(name="big", bufs=1))
        wpool = ctx.enter_context(tc.tile_pool(name="wpool", bufs=4))
        rowp = ctx.enter_context(tc.tile_pool(name="rowp", bufs=3))
        mpool = ctx.enter_context(tc.tile_pool(name="mpool", bufs=1))
        opool = ctx.enter_context(tc.tile_pool(name="opool", bufs=4))
        small = ctx.enter_context(tc.tile_pool(name="small", bufs=4))
        psP = ctx.enter_context(tc.tile_pool(name="psP", bufs=3, space="PSUM"))
        psD = ctx.enter_context(tc.tile_pool(name="psD", bufs=1, space="PSUM"))
        psB = ctx.enter_context(tc.tile_pool(name="psB", bufs=1, space="PSUM"))

        # ---- load inputs (xT chunks on the SP queue, weights on GpSimd's)
        xT_ch = []
        for c in range(NCHUNK):
            xc = big.tile([128, T], f16, tag=f"xt{c}")
            xT_ch.append(xc)
        for c in range(0, NCHUNK, 2):
            nc.sync.dma_start(out=xT_ch[c], in_=xT[c])
        wq_sb = big.tile([128, NCHUNK, HL * HD], f16, tag="wq")
        for c in range(NCHUNK):
            nc.gpsimd.dma_start(out=wq_sb[:, c, :], in_=wq[c])
        for c in range(1, NCHUNK, 2):
            nc.gpsimd.dma_start(out=xT_ch[c], in_=xT[c])
        w_sbs = []
        for name, srct in (("wk", wk), ("wv", wv)):
            sb = big.tile([128, NCHUNK, HL * HD], f16, tag=name)
            for c in range(NCHUNK):
                nc.sync.dma_start(out=sb[:, c, :], in_=srct[c])
            w_sbs.append(sb)
        wk_sb, wv_sb = w_sbs
        wp_sb = big.tile([128, NP, C], f16)
        for kc in range(NP):
            nc.gpsimd.dma_start(out=wp_sb[:, kc, :], in_=wp[kc])
        negm_sb = const.tile([128, 128], f16)
        nc.gpsimd.dma_start(out=negm_sb, in_=negm[:, :])
        idn16_sb = const.tile([128, 128], f16)
        nc.gpsimd.dma_start(out=idn16_sb, in_=idn16[:, :])
        idn32_sb = const.tile([128, 128], f32)
        nc.gpsimd.dma_start(out=idn32_sb, in_=idn32[:, :])
        ones_k1 = const.tile([1, 64], f16)
        nc.vector.memset(ones_k1, 1.0)
        ones_k64 = const.tile([64, 1], f16)
        nc.vector.memset(ones_k64, 1.0)

        # ---- persistent intermediates (per head-pair, for fine-grained deps)
        qT_p, kT_p, vT_p = [], [], []
        for pp in range(NP):
            tq = big.tile([128, T], f16, tag=f"q{pp}")
            tk = big.tile([128, T], f16, tag=f"k{pp}")
            tv = big.tile([128, T], f16, tag=f"v{pp}")  # becomes attn^T
            qT_p.append(tq)
            kT_p.append(tk)
            vT_p.append(tv)
        S_sb = big.tile([128, HL, 16], f32)   # row sums (column form)
        R_sb = big.tile([128, HL, 16], f32)   # 1/S (column form)

        # ---- phases 1+2 interleaved per head-pair:
        # proj(pair) -> heads 2p, 2p+1 (diag row, logits/exp/sums,
        # normalize, band DMA); per-head tails (R->row form, diag-weight
        # broadcast onto vT) are deferred two heads to keep the PE stream
        # free of fresh cross-engine dependencies.
        pending_tails = []

        def emit_tail(h, p, o, Erow):
            pst = psD.tile([16, 128], f32, tag="dg")
            nc.tensor.transpose(pst, R_sb[:, h, :], idn32_sb)
            et = small.tile([16, 128], f16, tag="et")
            nc.vector.tensor_copy(out=et, in_=pst)
            Rrow = rowp.tile([1, T], f16, tag="rrow")
            nc.sync.dma_start(
                out=Rrow.rearrange("p (b u) -> p b u", u=128), in_=et
            )
            rfac = rowp.tile([1, T], f16, tag="rfac")
            nc.vector.tensor_mul(rfac, Erow, Rrow)
            for jt in range(NT):
                bc = psB.tile([64, 512], f32, tag="bc")
                nc.tensor.matmul(
                    bc, ones_k1, rfac[0:1, jt * 512 : (jt + 1) * 512],
                    start=True, stop=True,
                )
                vs = vT_p[p][o : o + 64, jt * 512 : (jt + 1) * 512]
                nc.vector.tensor_tensor(out=vs, in0=vs, in1=bc, op=MULT)

        def head_block(h, fillers=()):
            fillers = list(fillers)
            p, o = h // 2, (h % 2) * 64
            kTh = kT_p[p][o : o + 64, :]
            qTh = qT_p[p][o : o + 64, :]

            # diag logits l_tt = sum_d qT[d,t]*kT[d,t]: elementwise + ones-mm
            m16 = mpool.tile([64, T], f16)
            nc.vector.tensor_mul(m16, qTh, kTh)
            Erow = rowp.tile([1, T], f16, tag="erow")
            for jt in range(NT):
                dg = psD.tile([1, 512], f32, tag="dg")
                nc.tensor.matmul(
                    dg, ones_k64, m16[:, jt * 512 : (jt + 1) * 512],
                    start=True, stop=True,
                )
                nc.scalar.activation(
                    out=Erow[0:1, jt * 512 : (jt + 1) * 512], in_=dg,
                    func=EXP, scale=SCALE,
                )

            def run_band(i):
                edge = (i + 1) * 128
                kblk = kTh[:, i * 128 : edge]
                wt = wpool.tile([128, T], f16, tag="wt")
                nslab = (edge + 1023) // 1024
                if nslab > 1:
                    st = small.tile([128, 2], f32, tag="st")
                else:
                    st = None
                for s in range(nslab):
                    base = 1024 * s
                    w = min(1024, edge - base)
                    ps = psP.tile([128, 1024], f32, tag="slab")
                    ccol = 0
                    while ccol < w:
                        cw = min(512, w - ccol)
                        last_chunk = base + ccol + cw == edge
                        nc.tensor.matmul(
                            ps[:, ccol : ccol + cw],
                            kblk,
                            qTh[:, base + ccol : base + ccol + cw],
                            start=True,
                            stop=not last_chunk,
                        )
                        ccol += cw
                    if base + w == edge:  # slab holding the diagonal block
                        nc.tensor.matmul(
                            ps[:, w - 128 : w], idn16_sb, negm_sb,
                            start=False, stop=True,
                        )
                    acc = (
                        S_sb[:, h, i : i + 1] if nslab == 1 else st[:, s : s + 1]
                    )
                    nc.scalar.activation(
                        out=wt[:, base : base + w], in_=ps[:, :w], func=EXP,
                        scale=SCALE, accum_out=acc,
                    )
                if nslab > 1:
                    nc.vector.tensor_reduce(
                        out=S_sb[:, h, i : i + 1], in_=st, axis=AXX, op=ADD
                    )
                return wt

            for i in range(16):
                wt = run_band(i)
                if fillers:
                    fillers.pop(0)()
                edge = (i + 1) * 128
                nc.vector.reciprocal(
                    out=R_sb[:, h, i : i + 1], in_=S_sb[:, h, i : i + 1]
                )
                nc.vector.tensor_scalar_mul(
                    wt[:, :edge], wt[:, :edge], R_sb[:, h, i : i + 1]
                )
                eng = nc.sync if i % 2 == 0 else nc.gpsimd
                eng.dma_start(
                    out=w_out[h, i * 128 : edge, 0:edge], in_=wt[:, :edge]
                )

            pending_tails.append((h, p, o, Erow))
            if len(pending_tails) > 2:
                emit_tail(*pending_tails.pop(0))

        def proj_group(p, wi, jt):
            wsb = (wq_sb, wk_sb, wv_sb)[wi]
            dstl = (qT_p, kT_p, vT_p)[wi]
            ps = psP.tile([128, 1024], f32, tag="slab")
            for c in range(NCHUNK):
                nc.tensor.matmul(
                    ps[:, :512],
                    wsb[:, c, p * 128 : (p + 1) * 128],
                    xT_ch[c][:, jt * 512 : (jt + 1) * 512],
                    start=(c == 0),
                    stop=(c == NCHUNK - 1),
                )
            dslice = dstl[p][:, jt * 512 : (jt + 1) * 512]
            nc.vector.tensor_copy(out=dslice, in_=ps[:, :512])

        def proj_pair(p):
            for wi in range(3):
                for jt in range(NT):
                    proj_group(p, wi, jt)

        import functools

        # pair 0: q/k up front; its v-projection flows into head 0 as fillers
        for wi in (0, 1):
            for jt in range(NT):
                proj_group(0, wi, jt)
        v0 = [functools.partial(proj_group, 0, 2, jt) for jt in range(NT)]
        for p in range(NP):
            if p + 1 < NP:
                groups = [
                    functools.partial(proj_group, p + 1, wi, jt)
                    for wi in range(3)
                    for jt in range(NT)
                ]
                head_block(2 * p, fillers=v0 + groups[:6])
                head_block(2 * p + 1, fillers=groups[6:])
                v0 = []
            else:
                head_block(2 * p)
                head_block(2 * p + 1)
        while pending_tails:
            emit_tail(*pending_tails.pop(0))

        # ---- phase 4: output projection (partial over this core's heads)
        for i in range(16):
            for n in range(2):
                if n == 0:
                    ps = psP.tile([128, 1024], f32, tag="slab")
                else:
                    ps = psB.tile([128, 512], f32, tag="bc")
                for kc in range(NP):
                    nc.tensor.matmul(
                        ps[:, :512],
                        vT_p[kc][:, i * 128 : (i + 1) * 128],
                        wp_sb[:, kc, n * 512 : (n + 1) * 512],
                        start=(kc == 0),
                        stop=(kc == NP - 1),
                    )
                ost = opool.tile([128, 512], f32, tag="ost")
                if n == 0:
                    nc.scalar.activation(out=ost, in_=ps[:, :512], func=COPY)
                else:
                    nc.vector.tensor_copy(out=ost, in_=ps[:, :512])
                eng = nc.sync if n == 0 else nc.gpsimd
                eng.dma_start(
                    out=o_out[i * 128 : (i + 1) * 128, n * 512 : (n + 1) * 512],
                    in_=ost,
                )

    nc.compile()
    return nc


def _get_nc():
    if "nc" not in _CACHE:
        _CACHE["nc"] = _build()
    return _CACHE["nc"]


_NEGM = np.triu(np.full((128, 128), NEG, np.float32), 1).astype(np.float16)
_IDN16 = np.eye(128, dtype=np.float16)
_IDN32 = np.eye(128, dtype=np.float32)


def _pack_w(W, h0):
    # W [H, C, HD] -> [NCHUNK, 128, HL*HD] with cols = h_local*HD + d
    A = W[h0 : h0 + HL].transpose(1, 0, 2).reshape(C, HL * HD)
    return np.ascontiguousarray(A).reshape(NCHUNK, 128, HL * HD).astype(np.float16)


def make_in_maps(x, Wq, Wk, Wv, Wp):
    in_maps = []
    for core in range(N_CORES):
        b, g = core // G, core % G
        h0 = g * HL
        xb = np.ascontiguousarray(x[b].T).reshape(NCHUNK, 128, T).astype(np.float16)
        wp_pack = (
            np.ascontiguousarray(Wp[:, g * HL * HD : (g + 1) * HL * HD].T)
            .reshape(NP, 128, C)
            .astype(np.float16)
        )
        in_maps.append(
            {
                "xT": xb,
                "wq": _pack_w(Wq, h0),
                "wk": _pack_w(Wk, h0),
                "wv": _pack_w(Wv, h0),
                "wp": wp_pack,
                "negm": _NEGM,
                "idn16": _IDN16,
                "idn32": _IDN32,
            }
        )
    return in_maps


def assemble(results, bp):
    weights = np.empty((B, H, T, T), np.float32)
    out = np.empty((B, T, C), np.float32)
    for b in range(B):
        weights[b, :HL] = results[G * b]["w_out"].astype(np.float32)
        weights[b, HL:] = results[G * b + 1]["w_out"].astype(np.float32)
        out[b] = results[G * b]["o_out"] + results[G * b + 1]["o_out"] + bp[None, :]
    return out, weights


def kernel(**inputs):
    x = np.asarray(inputs["x"], np.float32)
    Wq = np.asarray(inputs["Wq"], np.float32)
    Wk = np.asarray(inputs["Wk"], np.float32)
    Wv = np.asarray(inputs["Wv"], np.float32)
    Wp = np.asarray(inputs["Wp"], np.float32)
    bp = np.asarray(inputs["bp"], np.float32)
    nc = _get_nc()
    in_maps = make_in_maps(x, Wq, Wk, Wv, Wp)
    res = run_bass_kernel_spmd(nc, in_maps, core_ids=list(range(N_CORES)))
    return assemble(res.results, bp)
